# revision 40
# baseline (speedup 1.0000x reference)
"""Multi-head self-attention (B=2, L=2048, D=1024, H=16) on 8 TRN2 NeuronCores.

Sharding: core c -> (batch b = c//4, head-group g = c%4 of 4 heads).
Each core computes, for its batch element and its 4 heads:
  qkv projection (column-sharded), scores, softmax, attn@V, and the
  row-sharded slice of the output projection (partial sums over D).
Host gathers: sums the 4 partial outputs per batch and transposes.

Device-side design:
  - x is passed pre-transposed (xT [D, L]); q^T and k^T are computed
    directly ([c, L], partition = head channel) so scores^T [k_l, q_l]
    come out of the PE in one pass.
  - ALL matmuls keep K=128: the per-head S^T matmul (d=64) uses a
    zero-padded lhsT (kT2[h]: the head's 64 k-channel rows at their
    natural partition offset, zeros in the other 64 rows) against the
    full 128-partition q^T pair tile.  K<128 matmuls keep the PE's HAM
    clock-gate cold (measured 435 vs 236 ns for N=512) — zero-padding
    keeps the array fully loaded and the clock at 2.4 GHz.
  - exp() without max-subtraction (scores ~N(0,1) after the 1/8 scale,
    folded into the activation scale), over merged [128,1024] tiles to
    amortize ACT instruction overhead.
  - v is augmented with a ones column, so attn@V also yields the
    softmax denominator as row 64 of ctx^T for free.
  - ctx_aug^T [65, q] is PE-transposed (zero-padded to K=128) to
    [q, 65]; normalization is a per-partition reciprocal-scale on DVE.
  - out^T [D, L] = W_out-shard^T @ ctx^T; host transposes + reduces.
  - matmuls run in float32r (full-speed fp32 mode, ~12-bit mantissa);
    every matmul input is produced as f32r (DVE/ACT round on write).
  - q/k biases applied on-device at eviction; the v bias equals adding
    (b_v @ W_out) to the final output (softmax rows sum to 1): host.
"""

import numpy as np
from contextlib import ExitStack

import concourse.bacc as bacc
import concourse.bass as bass
import concourse.tile as tile
from concourse import mybir
from concourse.bass import ts
from concourse.bass_utils import run_bass_kernel_spmd
from concourse.masks import make_identity

# Problem constants (hardcoded per the self-contained-kernel contract).
B, L, D, H, HD = 2, 2048, 1024, 16, 64
N_CORES = 8
GROUPS = 4                  # head-groups per batch element
HPC = H // GROUPS           # heads per core = 4
CS = HPC * HD               # channel shard = 256
P = 128
KT = D // P                 # 8 k-tiles over D
NL = L // 512               # 4 l-chunks of 512
LT = L // P                 # 16 l-tiles of 128
CT_QK = 2 * CS // P         # 4 c-tiles over [q|k] shard (512)

F32 = mybir.dt.float32
F32R = mybir.dt.float32r
USE_F32R = True
MDT = F32R if USE_F32R else F32
Ident = mybir.ActivationFunctionType.Identity
Exp = mybir.ActivationFunctionType.Exp

_NC_CACHE = {}


def _build_body(nc, ctx, tc, xT, w_qk, w_v, b_qk, w_out, outT):
    const = ctx.enter_context(tc.tile_pool(name="const", bufs=1))

    ones_sb = const.tile([P, 1], F32, tag="ones")
    nc.vector.memset(ones_sb[:], 1.0)
    zeros_sb = const.tile([P, 512], F32, tag="zeros")
    nc.vector.memset(zeros_sb[:], 0.0)
    ident = const.tile([P, P], F32, tag="ident")
    make_identity(nc, ident)

    wout_sb = [const.tile([P, D], MDT, tag=f"wout{t}", name=f"wout{t}")
               for t in range(CS // P)]
    bqk_sb = [const.tile([P, 1], F32, tag=f"bqk{m}", name=f"bqk{m}")
              for m in range(CT_QK)]
    # q^T pair tiles: rows 0:64 head 2p, 64:128 head 2p+1
    qT_sb = [const.tile([P, L], MDT, tag=f"qT{p}", name=f"qT{p}") for p in range(2)]
    # zero-padded k^T per head: head rows at natural offset, other half 0
    kT2_sb = [const.tile([P, L], MDT, tag=f"kT2{h}", name=f"kT2{h}") for h in range(HPC)]
    # v_aug per l-tile: per head [v(64) | ones] (65 cols)
    VOFF = [65 * h for h in range(HPC)]
    VTOT = HPC * (HD + 1)
    v_sb = [const.tile([P, VTOT], MDT, tag=f"v{t}", name=f"v{t}") for t in range(LT)]
    ctx_sb = [const.tile([P, CS], F32, tag=f"ctx{t}", name=f"ctx{t}")
              for t in range(LT)]
    cxT_sb = [const.tile([P, L], MDT, tag=f"cxT{t}", name=f"cxT{t}")
              for t in range(CS // P)]

    # stage-1-scoped pools (released after stage 1)
    s1 = ExitStack()
    s1pool = s1.enter_context(tc.tile_pool(name="s1w", bufs=1))
    stage = s1.enter_context(tc.tile_pool(name="stage", bufs=3))
    xpool = s1.enter_context(tc.tile_pool(name="xt", bufs=2))

    pspool = ctx.enter_context(tc.tile_pool(name="ps", bufs=3, space="PSUM"))
    accpool = ctx.enter_context(tc.tile_pool(name="acc", bufs=2, space="PSUM"))

    def load_rounded(dst, src_slice, nm):
        if USE_F32R:
            st = stage.tile(list(dst.shape), F32, tag="st", name=f"st_{nm}")
            nc.sync.dma_start(st[:], src_slice)
            nc.vector.tensor_copy(dst, st[:])
        else:
            nc.sync.dma_start(dst, src_slice)

    wqk_sb = [s1pool.tile([P, 2 * CS], MDT, tag=f"wqk{k}", name=f"wqk{k}")
              for k in range(KT)]
    wv_sb = [s1pool.tile([P, CS], MDT, tag=f"wv{k}", name=f"wv{k}")
             for k in range(KT)]
    for k in range(KT):
        load_rounded(wqk_sb[k][:], w_qk[ts(k, P), :], f"wqk{k}")
        load_rounded(wv_sb[k][:], w_v[ts(k, P), :], f"wv{k}")
    for t in range(CS // P):
        load_rounded(wout_sb[t][:, 0:512], w_out[ts(t, P), 0:512], f"wouta{t}")
        load_rounded(wout_sb[t][:, 512:1024], w_out[ts(t, P), 512:1024], f"woutb{t}")
    for m in range(CT_QK):
        nc.sync.dma_start(bqk_sb[m][:], b_qk[ts(m, P), :])

    # constant fills for kT2 zero halves (gate the first S matmuls — early)
    for h in range(HPC):
        zr = slice(64, 128) if h % 2 == 0 else slice(0, 64)
        for lc in range(NL):
            nc.vector.tensor_copy(kT2_sb[h][zr, ts(lc, 512)], zeros_sb[0:64, :])

    # ---- Stage 1: qkv projections (all share the x tiles) ----------------
    for lc in range(NL):
        xts = []
        for k in range(KT):
            xt = xpool.tile([P, 512], MDT, tag=f"x{k}", name=f"x{k}_{lc}")
            load_rounded(xt[:], xT[ts(k, P), ts(lc, 512)], f"x{k}_{lc}")
            xts.append(xt)
        for m in range(CT_QK):
            ps = pspool.tile([P, 512], F32, tag="ps", name=f"qk_ps{lc}_{m}")
            for k in range(KT):
                nc.tensor.matmul(ps[:], wqk_sb[k][:, ts(m, P)], xts[k][:],
                                 start=(k == 0), stop=(k == KT - 1))
            if m < 2:
                nc.scalar.activation(qT_sb[m][:, ts(lc, 512)], ps[:], Ident,
                                     bias=bqk_sb[m][:])
            else:
                p = m - 2
                nc.scalar.activation(kT2_sb[2 * p][0:64, ts(lc, 512)], ps[0:64, :],
                                     Ident, bias=bqk_sb[m][0:64, :])
                nc.scalar.activation(kT2_sb[2 * p + 1][64:128, ts(lc, 512)],
                                     ps[64:128, :], Ident, bias=bqk_sb[m][64:128, :])
        for i in range(4):
            t = lc * 4 + i
            vps = accpool.tile([P, CS], F32, tag="acc", name=f"v_ps{t}")
            for k in range(KT):
                nc.tensor.matmul(vps[:], xts[k][:, ts(i, P)], wv_sb[k][:],
                                 start=(k == 0), stop=(k == KT - 1))
            for h in range(HPC):
                nc.vector.tensor_copy(v_sb[t][:, VOFF[h]:VOFF[h] + HD],
                                      vps[:, ts(h, HD)])

    # v ones columns (only needed by the PV matmuls, per l-tile)
    for t in range(LT):
        for h in range(HPC):
            nc.vector.tensor_copy(v_sb[t][:, VOFF[h] + HD:VOFF[h] + HD + 1],
                                  ones_sb[:])
    s1.close()

    ptpool = ctx.enter_context(tc.tile_pool(name="pt", bufs=3))
    capool = ctx.enter_context(tc.tile_pool(name="ca", bufs=10))
    rpool = ctx.enter_context(tc.tile_pool(name="r", bufs=8))
    opool = ctx.enter_context(tc.tile_pool(name="ot", bufs=4))

    # ---- Stage 2: attention (qg outer); epilogue interleaved --------------
    pending = []   # deferred out-proj emission units, drained 1/kt-iter

    def drain(n):
        for _ in range(min(n, len(pending))):
            pending.pop(0)()

    def outproj_unit(et, lc):
        def emit():
            ops = pspool.tile([P, 512], F32, tag="ps", name=f"o_ps{et}_{lc}")
            for ct in range(CS // P):
                nc.tensor.matmul(ops[:], wout_sb[ct][:, ts(et, P)],
                                 cxT_sb[ct][:, ts(lc, 512)],
                                 start=(ct == 0), stop=(ct == CS // P - 1))
            ot = opool.tile([P, 512], F32, tag="ot", name=f"ot{et}_{lc}")
            nc.vector.tensor_copy(ot[:], ops[:])
            nc.sync.dma_start(outT[ts(et, P), ts(lc, 512)], ot[:])
        return emit

    def tn_unit(h, qc, ca):
        # transpose one 128-block of ctx_aug^T and normalize it into ctx
        def emit():
            tp = pspool.tile([P, 512], F32, tag="ps", name=f"tp{h}_{qc}")
            for i in range(4):
                t = qc * 4 + i
                nc.tensor.transpose(tp[:, ts(i, P)], ca[:, ts(i, P)], ident[:])
            for i in range(4):
                t = qc * 4 + i
                r = rpool.tile([P, 1], F32, tag="r", name=f"r{h}_{qc}_{i}")
                nc.vector.reciprocal(r[:], tp[:, i * P + HD:i * P + HD + 1])
                nc.vector.tensor_scalar_mul(ctx_sb[t][:, ts(h, HD)],
                                            tp[:, i * P:i * P + HD], r[:])
        return emit

    def tp2_unit(qg, ct, j):
        def emit():
            t = 8 * qg + j
            tp2 = pspool.tile([P, 512], F32, tag="ps", name=f"tp2_{ct}_{t}")
            nc.tensor.transpose(tp2[:, 0:P], ctx_sb[t][:, ts(ct, P)], ident[:])
            nc.vector.tensor_copy(cxT_sb[ct][:, ts(t, P)], tp2[:, 0:P])
        return emit

    ca_first = [True] * 10   # zero rows 64:128 on first use of each ca slot
    for qg in range(NL // 2):
        for h in range(HPC):
            vw = HD + 1
            cps = [accpool.tile([P, 512], F32, tag="acc", name=f"ctx_ps{h}_{qg}_{i}")
                   for i in range(2)]
            prev = None
            for kt in range(LT):
                sps = pspool.tile([P, 1024], F32, tag="ps", name=f"s_ps{h}_{qg}_{kt}")
                for half in range(2):
                    nc.tensor.matmul(sps[:, half * 512:(half + 1) * 512],
                                     kT2_sb[h][:, ts(kt, P)],
                                     qT_sb[h // 2][:, ts(2 * qg + half, 512)],
                                     start=True, stop=True)
                pt = ptpool.tile([P, 1024], MDT, tag="pt", name=f"pt{h}_{qg}_{kt}")
                nc.scalar.activation(pt[:], sps[:], Exp, scale=1.0 / np.sqrt(HD))
                if prev is not None:
                    for half in range(2):
                        nc.tensor.matmul(
                            cps[half][0:vw, :],
                            v_sb[prev][:, VOFF[h]:VOFF[h] + vw],
                            prevpt[:, half * 512:(half + 1) * 512],
                            start=(prev == 0), stop=False)
                prev, prevpt = kt, pt
                drain(1)
            for half in range(2):
                nc.tensor.matmul(cps[half][0:vw, :],
                                 v_sb[prev][:, VOFF[h]:VOFF[h] + vw],
                                 prevpt[:, half * 512:(half + 1) * 512],
                                 start=False, stop=True)
            # evict ctx_aug^T to SBUF; defer transpose+normalize
            for half in range(2):
                qc = 2 * qg + half
                ca = capool.tile([P, 512], F32, tag="ca", name=f"ca{h}_{qc}")
                if ca_first:
                    ca_first.pop()
                    nc.vector.tensor_copy(ca[HD:P, :], zeros_sb[HD:P, :])
                nc.vector.tensor_copy(ca[0:HD + 1, :], cps[half][0:HD + 1, :])
                pending.append(tn_unit(h, qc, ca))

        # queue ctx -> ctx^T transposes and out-projection for this q-range
        for ct in range(CS // P):
            for j in range(8):
                pending.append(tp2_unit(qg, ct, j))
        for et in range(D // P):
            for lc in range(2 * qg, 2 * (qg + 1)):
                pending.append(outproj_unit(et, lc))
    drain(len(pending))


def build_nc():
    key = ("v9", USE_F32R)
    if key in _NC_CACHE:
        return _NC_CACHE[key]
    nc = bacc.Bacc("TRN2", target_bir_lowering=False, debug=False)
    xT = nc.dram_tensor("xT", [D, L], F32, kind="ExternalInput").ap()
    w_qk = nc.dram_tensor("w_qk", [D, 2 * CS], F32, kind="ExternalInput").ap()
    w_v = nc.dram_tensor("w_v", [D, CS], F32, kind="ExternalInput").ap()
    b_qk = nc.dram_tensor("b_qk", [2 * CS, 1], F32, kind="ExternalInput").ap()
    w_out = nc.dram_tensor("w_out", [CS, D], F32, kind="ExternalInput").ap()
    outT = nc.dram_tensor("outT", [D, L], F32, kind="ExternalOutput").ap()
    with tile.TileContext(nc) as tc:
        with ExitStack() as ctx:
            _build_body(nc, ctx, tc, xT, w_qk, w_v, b_qk, w_out, outT)
    nc.compile()
    _NC_CACHE[key] = nc
    return nc


def make_in_maps(x, W_qkv, b_qkv, W_out):
    x = np.ascontiguousarray(np.asarray(x, dtype=np.float32))
    W_qkv = np.asarray(W_qkv, dtype=np.float32)
    b_qkv = np.asarray(b_qkv, dtype=np.float32)
    W_out = np.asarray(W_out, dtype=np.float32)
    Wq, Wk, Wv = W_qkv[:, 0:D], W_qkv[:, D:2 * D], W_qkv[:, 2 * D:3 * D]
    bq, bk = b_qkv[0:D], b_qkv[D:2 * D]
    in_maps = []
    xTs = [np.ascontiguousarray(x[b].T) for b in range(B)]
    for c in range(N_CORES):
        b, g = divmod(c, GROUPS)
        cs = slice(CS * g, CS * (g + 1))
        in_maps.append({
            "xT": xTs[b],
            "w_qk": np.ascontiguousarray(np.concatenate([Wq[:, cs], Wk[:, cs]], axis=1)),
            "w_v": np.ascontiguousarray(Wv[:, cs]),
            "b_qk": np.ascontiguousarray(
                np.concatenate([bq[cs], bk[cs]]).reshape(2 * CS, 1)),
            "w_out": np.ascontiguousarray(W_out[cs, :]),
        })
    return in_maps


def combine_outputs(results, b_qkv, b_out, W_out):
    b_qkv = np.asarray(b_qkv, dtype=np.float32)
    b_out = np.asarray(b_out, dtype=np.float32)
    W_out = np.asarray(W_out, dtype=np.float32)
    out = np.empty((B, L, D), np.float32)
    for b in range(B):
        acc = results[GROUPS * b]["outT"].astype(np.float32)
        for g in range(1, GROUPS):
            acc = acc + results[GROUPS * b + g]["outT"]
        out[b] = acc.T
    # v-bias folds to a constant row (softmax rows sum to 1); plus b_out.
    bv = b_qkv[2 * D:3 * D]
    out += (bv @ W_out + b_out)[None, None, :]
    return out


def _numpy_reference(x, attention_mask, W_qkv, b_qkv, W_out, b_out):
    x = np.asarray(x, np.float64)
    mask = np.asarray(attention_mask, bool)
    W_qkv = np.asarray(W_qkv, np.float64)
    b_qkv = np.asarray(b_qkv, np.float64)
    W_out = np.asarray(W_out, np.float64)
    b_out = np.asarray(b_out, np.float64)
    Bs, Ls, Ds = x.shape
    qkv = x @ W_qkv + b_qkv
    qkv = qkv.reshape(Bs, Ls, 3, H, HD)
    q = np.transpose(qkv[:, :, 0], (0, 2, 1, 3))
    k = np.transpose(qkv[:, :, 1], (0, 2, 1, 3))
    v = np.transpose(qkv[:, :, 2], (0, 2, 1, 3))
    scores = np.einsum("bhqd,bhkd->bhqk", q, k) / np.sqrt(HD)
    scores = np.where(~mask[:, None, None, :], -np.inf, scores)
    scores = scores - scores.max(axis=-1, keepdims=True)
    attn = np.exp(scores)
    attn = attn / attn.sum(axis=-1, keepdims=True)
    ctx = np.einsum("bhqk,bhkd->bhqd", attn, v)
    ctx = np.transpose(ctx, (0, 2, 1, 3)).reshape(Bs, Ls, Ds)
    return (ctx @ W_out + b_out).astype(np.float32)


def kernel(x, attention_mask, W_qkv, b_qkv, W_out, b_out):
    mask = np.asarray(attention_mask, bool)
    if not mask.all():
        return _numpy_reference(x, attention_mask, W_qkv, b_qkv, W_out, b_out)
    nc = build_nc()
    in_maps = make_in_maps(x, W_qkv, b_qkv, W_out)
    res = run_bass_kernel_spmd(nc, in_maps, list(range(N_CORES)))
    return combine_outputs(res.results, b_qkv, b_out, W_out)


# revision 43
# speedup vs baseline: 1.0273x; 1.0273x over previous
"""Multi-head self-attention (B=2, L=2048, D=1024, H=16) on 8 TRN2 NeuronCores.

Sharding: core c -> (batch b = c//4, head-group g = c%4 of 4 heads).
Each core computes, for its batch element and its 4 heads:
  qkv projection (column-sharded), scores, softmax, attn@V, and the
  row-sharded slice of the output projection (partial sums over D).
Host gathers: sums the 4 partial outputs per batch and transposes.

Device-side design:
  - x is passed pre-transposed (xT [D, L]); q^T and k^T are computed
    directly ([c, L], partition = head channel) so scores^T [k_l, q_l]
    come out of the PE in one pass.
  - ALL matmuls keep K=128: the per-head S^T matmul (d=64) uses a
    zero-padded lhsT (kT2[h]: the head's 64 k-channel rows at their
    natural partition offset, zeros in the other 64 rows) against the
    full 128-partition q^T pair tile.  K<128 matmuls keep the PE's HAM
    clock-gate cold (measured 435 vs 236 ns for N=512) — zero-padding
    keeps the array fully loaded and the clock at 2.4 GHz.
  - exp() without max-subtraction (scores ~N(0,1) after the 1/8 scale,
    folded into the activation scale), over merged [128,1024] tiles to
    amortize ACT instruction overhead.
  - v is augmented with a ones column, so attn@V also yields the
    softmax denominator as row 64 of ctx^T for free.
  - ctx_aug^T [65, q] is PE-transposed (zero-padded to K=128) to
    [q, 65]; normalization is a per-partition reciprocal-scale on DVE.
  - out^T [D, L] = W_out-shard^T @ ctx^T; host transposes + reduces.
  - matmuls run in float32r (full-speed fp32 mode, ~12-bit mantissa);
    every matmul input is produced as f32r (DVE/ACT round on write).
  - q/k biases applied on-device at eviction; the v bias equals adding
    (b_v @ W_out) to the final output (softmax rows sum to 1): host.
"""

import numpy as np
from contextlib import ExitStack

import concourse.bacc as bacc
import concourse.bass as bass
import concourse.tile as tile
from concourse import mybir
from concourse.bass import ts
from concourse.bass_utils import run_bass_kernel_spmd
from concourse.masks import make_identity

# Problem constants (hardcoded per the self-contained-kernel contract).
B, L, D, H, HD = 2, 2048, 1024, 16, 64
N_CORES = 8
GROUPS = 4                  # head-groups per batch element
HPC = H // GROUPS           # heads per core = 4
CS = HPC * HD               # channel shard = 256
P = 128
KT = D // P                 # 8 k-tiles over D
NL = L // 512               # 4 l-chunks of 512
LT = L // P                 # 16 l-tiles of 128
CT_QK = 2 * CS // P         # 4 c-tiles over [q|k] shard (512)

F32 = mybir.dt.float32
F32R = mybir.dt.float32r
USE_F32R = True
MDT = F32R if USE_F32R else F32
Ident = mybir.ActivationFunctionType.Identity
Exp = mybir.ActivationFunctionType.Exp

_NC_CACHE = {}


def _build_body(nc, ctx, tc, xT, w_qk, w_v, b_qk, w_out, outT):
    const = ctx.enter_context(tc.tile_pool(name="const", bufs=1))

    ones_sb = const.tile([P, 1], F32, tag="ones")
    nc.vector.memset(ones_sb[:], 1.0)
    zeros_sb = const.tile([P, 512], F32, tag="zeros")
    nc.vector.memset(zeros_sb[:], 0.0)
    ident = const.tile([P, P], F32, tag="ident")
    make_identity(nc, ident)

    wout_sb = [const.tile([P, D], MDT, tag=f"wout{t}", name=f"wout{t}")
               for t in range(CS // P)]
    bqk_sb = [const.tile([P, 1], F32, tag=f"bqk{m}", name=f"bqk{m}")
              for m in range(CT_QK)]
    # q^T pair tiles: rows 0:64 head 2p, 64:128 head 2p+1
    qT_sb = [const.tile([P, L], MDT, tag=f"qT{p}", name=f"qT{p}") for p in range(2)]
    # zero-padded k^T per head: head rows at natural offset, other half 0
    kT2_sb = [const.tile([P, L], MDT, tag=f"kT2{h}", name=f"kT2{h}") for h in range(HPC)]
    # v_aug per l-tile: per head [v(64) | ones] (65 cols)
    VOFF = [65 * h for h in range(HPC)]
    VTOT = HPC * (HD + 1)
    v_sb = [const.tile([P, VTOT], MDT, tag=f"v{t}", name=f"v{t}") for t in range(LT)]
    ctx_sb = [const.tile([P, CS], F32, tag=f"ctx{t}", name=f"ctx{t}")
              for t in range(LT)]
    cxT_sb = [const.tile([P, L], MDT, tag=f"cxT{t}", name=f"cxT{t}")
              for t in range(CS // P)]

    # stage-1-scoped pools (released after stage 1)
    s1 = ExitStack()
    s1pool = s1.enter_context(tc.tile_pool(name="s1w", bufs=1))
    stage = s1.enter_context(tc.tile_pool(name="stage", bufs=3))
    xpool = s1.enter_context(tc.tile_pool(name="xt", bufs=2))

    pspool = ctx.enter_context(tc.tile_pool(name="ps", bufs=3, space="PSUM"))
    accpool = ctx.enter_context(tc.tile_pool(name="acc", bufs=2, space="PSUM"))

    def load_rounded(dst, src_slice, nm):
        if USE_F32R:
            st = stage.tile(list(dst.shape), F32, tag="st", name=f"st_{nm}")
            nc.sync.dma_start(st[:], src_slice)
            nc.vector.tensor_copy(dst, st[:])
        else:
            nc.sync.dma_start(dst, src_slice)

    wqk_sb = [s1pool.tile([P, 2 * CS], MDT, tag=f"wqk{k}", name=f"wqk{k}")
              for k in range(KT)]
    wv_sb = [s1pool.tile([P, CS], MDT, tag=f"wv{k}", name=f"wv{k}")
             for k in range(KT)]

    # interleave the first l-chunk's x tiles with the qk weights so the
    # first psum accumulation chain can start within a few microseconds
    xts0 = []
    for k in range(KT):
        xt = xpool.tile([P, 512], MDT, tag=f"x{k}", name=f"x{k}_0")
        load_rounded(xt[:], xT[ts(k, P), 0:512], f"x{k}_0")
        xts0.append(xt)
        load_rounded(wqk_sb[k][:], w_qk[ts(k, P), :], f"wqk{k}")
    for m in range(CT_QK):
        nc.sync.dma_start(bqk_sb[m][:], b_qk[ts(m, P), :])
    for k in range(KT):
        load_rounded(wv_sb[k][:], w_v[ts(k, P), :], f"wv{k}")

    # constant fills for kT2 zero halves (gate the first S matmuls)
    for h in range(HPC):
        zr = slice(64, 128) if h % 2 == 0 else slice(0, 64)
        for lc in range(NL):
            nc.vector.tensor_copy(kT2_sb[h][zr, ts(lc, 512)], zeros_sb[0:64, :])

    # ---- Stage 1: qkv projections (all share the x tiles) ----------------
    for lc in range(NL):
        if lc == 0:
            xts = xts0
        else:
            xts = []
            for k in range(KT):
                xt = xpool.tile([P, 512], MDT, tag=f"x{k}", name=f"x{k}_{lc}")
                load_rounded(xt[:], xT[ts(k, P), ts(lc, 512)], f"x{k}_{lc}")
                xts.append(xt)
        for m in range(CT_QK):
            ps = pspool.tile([P, 512], F32, tag="ps", name=f"qk_ps{lc}_{m}")
            for k in range(KT):
                nc.tensor.matmul(ps[:], wqk_sb[k][:, ts(m, P)], xts[k][:],
                                 start=(k == 0), stop=(k == KT - 1))
            if m < 2:
                nc.scalar.activation(qT_sb[m][:, ts(lc, 512)], ps[:], Ident,
                                     bias=bqk_sb[m][:])
            else:
                p = m - 2
                nc.scalar.activation(kT2_sb[2 * p][0:64, ts(lc, 512)], ps[0:64, :],
                                     Ident, bias=bqk_sb[m][0:64, :])
                nc.scalar.activation(kT2_sb[2 * p + 1][64:128, ts(lc, 512)],
                                     ps[64:128, :], Ident, bias=bqk_sb[m][64:128, :])
        for i in range(4):
            t = lc * 4 + i
            vps = accpool.tile([P, CS], F32, tag="acc", name=f"v_ps{t}")
            for k in range(KT):
                nc.tensor.matmul(vps[:], xts[k][:, ts(i, P)], wv_sb[k][:],
                                 start=(k == 0), stop=(k == KT - 1))
            for h in range(HPC):
                # ACT is idle through most of stage 1; keep DVE on the casts
                nc.scalar.copy(v_sb[t][:, VOFF[h]:VOFF[h] + HD],
                               vps[:, ts(h, HD)])

    # wout loads (first needed by the deferred out-proj, much later)
    for t in range(CS // P):
        load_rounded(wout_sb[t][:, 0:512], w_out[ts(t, P), 0:512], f"wouta{t}")
        load_rounded(wout_sb[t][:, 512:1024], w_out[ts(t, P), 512:1024], f"woutb{t}")
    # v ones columns (only needed by the PV matmuls, per l-tile)
    for t in range(LT):
        for h in range(HPC):
            nc.vector.tensor_copy(v_sb[t][:, VOFF[h] + HD:VOFF[h] + HD + 1],
                                  ones_sb[:])
    s1.close()

    ptpool = ctx.enter_context(tc.tile_pool(name="pt", bufs=3))
    capool = ctx.enter_context(tc.tile_pool(name="ca", bufs=10))
    rpool = ctx.enter_context(tc.tile_pool(name="r", bufs=8))
    opool = ctx.enter_context(tc.tile_pool(name="ot", bufs=4))

    # ---- Stage 2: attention (qg outer); epilogue interleaved --------------
    pending = []   # deferred out-proj emission units, drained 1/kt-iter

    def drain(n):
        for _ in range(min(n, len(pending))):
            pending.pop(0)()

    def outproj_unit(et, lc):
        def emit():
            ops = pspool.tile([P, 512], F32, tag="ps", name=f"o_ps{et}_{lc}")
            for ct in range(CS // P):
                nc.tensor.matmul(ops[:], wout_sb[ct][:, ts(et, P)],
                                 cxT_sb[ct][:, ts(lc, 512)],
                                 start=(ct == 0), stop=(ct == CS // P - 1))
            ot = opool.tile([P, 512], F32, tag="ot", name=f"ot{et}_{lc}")
            nc.vector.tensor_copy(ot[:], ops[:])
            nc.sync.dma_start(outT[ts(et, P), ts(lc, 512)], ot[:])
        return emit

    def tn_unit(h, qc, ca):
        # transpose one 128-block of ctx_aug^T and normalize it into ctx
        def emit():
            tp = pspool.tile([P, 512], F32, tag="ps", name=f"tp{h}_{qc}")
            for i in range(4):
                t = qc * 4 + i
                nc.tensor.transpose(tp[:, ts(i, P)], ca[:, ts(i, P)], ident[:])
            for i in range(4):
                t = qc * 4 + i
                r = rpool.tile([P, 1], F32, tag="r", name=f"r{h}_{qc}_{i}")
                nc.vector.reciprocal(r[:], tp[:, i * P + HD:i * P + HD + 1])
                nc.vector.tensor_scalar_mul(ctx_sb[t][:, ts(h, HD)],
                                            tp[:, i * P:i * P + HD], r[:])
        return emit

    def tp2_unit(qg, ct, j):
        def emit():
            t = 8 * qg + j
            tp2 = pspool.tile([P, 512], F32, tag="ps", name=f"tp2_{ct}_{t}")
            nc.tensor.transpose(tp2[:, 0:P], ctx_sb[t][:, ts(ct, P)], ident[:])
            nc.vector.tensor_copy(cxT_sb[ct][:, ts(t, P)], tp2[:, 0:P])
        return emit

    ca_first = [True] * 10   # zero rows 64:128 on first use of each ca slot
    for qg in range(NL // 2):
        for h in range(HPC):
            vw = HD + 1
            cps = [accpool.tile([P, 512], F32, tag="acc", name=f"ctx_ps{h}_{qg}_{i}")
                   for i in range(2)]
            prev = None
            for kt in range(LT):
                sps = pspool.tile([P, 1024], F32, tag="ps", name=f"s_ps{h}_{qg}_{kt}")
                for half in range(2):
                    nc.tensor.matmul(sps[:, half * 512:(half + 1) * 512],
                                     kT2_sb[h][:, ts(kt, P)],
                                     qT_sb[h // 2][:, ts(2 * qg + half, 512)],
                                     start=True, stop=True)
                pt = ptpool.tile([P, 1024], MDT, tag="pt", name=f"pt{h}_{qg}_{kt}")
                nc.scalar.activation(pt[:], sps[:], Exp, scale=1.0 / np.sqrt(HD))
                if prev is not None:
                    for half in range(2):
                        nc.tensor.matmul(
                            cps[half][0:vw, :],
                            v_sb[prev][:, VOFF[h]:VOFF[h] + vw],
                            prevpt[:, half * 512:(half + 1) * 512],
                            start=(prev == 0), stop=False)
                prev, prevpt = kt, pt
                drain(1)
            for half in range(2):
                nc.tensor.matmul(cps[half][0:vw, :],
                                 v_sb[prev][:, VOFF[h]:VOFF[h] + vw],
                                 prevpt[:, half * 512:(half + 1) * 512],
                                 start=False, stop=True)
            # evict ctx_aug^T to SBUF; defer transpose+normalize
            for half in range(2):
                qc = 2 * qg + half
                ca = capool.tile([P, 512], F32, tag="ca", name=f"ca{h}_{qc}")
                if ca_first:
                    ca_first.pop()
                    nc.vector.tensor_copy(ca[HD:P, :], zeros_sb[HD:P, :])
                nc.vector.tensor_copy(ca[0:HD + 1, :], cps[half][0:HD + 1, :])
                pending.append(tn_unit(h, qc, ca))

        # queue ctx -> ctx^T transposes and out-projection for this q-range
        for ct in range(CS // P):
            for j in range(8):
                pending.append(tp2_unit(qg, ct, j))
        for et in range(D // P):
            for lc in range(2 * qg, 2 * (qg + 1)):
                pending.append(outproj_unit(et, lc))
    drain(len(pending))


def build_nc():
    key = ("v10", USE_F32R)
    if key in _NC_CACHE:
        return _NC_CACHE[key]
    nc = bacc.Bacc("TRN2", target_bir_lowering=False, debug=False)
    xT = nc.dram_tensor("xT", [D, L], F32, kind="ExternalInput").ap()
    w_qk = nc.dram_tensor("w_qk", [D, 2 * CS], F32, kind="ExternalInput").ap()
    w_v = nc.dram_tensor("w_v", [D, CS], F32, kind="ExternalInput").ap()
    b_qk = nc.dram_tensor("b_qk", [2 * CS, 1], F32, kind="ExternalInput").ap()
    w_out = nc.dram_tensor("w_out", [CS, D], F32, kind="ExternalInput").ap()
    outT = nc.dram_tensor("outT", [D, L], F32, kind="ExternalOutput").ap()
    with tile.TileContext(nc) as tc:
        with ExitStack() as ctx:
            _build_body(nc, ctx, tc, xT, w_qk, w_v, b_qk, w_out, outT)
    nc.compile()
    _NC_CACHE[key] = nc
    return nc


def make_in_maps(x, W_qkv, b_qkv, W_out):
    x = np.ascontiguousarray(np.asarray(x, dtype=np.float32))
    W_qkv = np.asarray(W_qkv, dtype=np.float32)
    b_qkv = np.asarray(b_qkv, dtype=np.float32)
    W_out = np.asarray(W_out, dtype=np.float32)
    Wq, Wk, Wv = W_qkv[:, 0:D], W_qkv[:, D:2 * D], W_qkv[:, 2 * D:3 * D]
    bq, bk = b_qkv[0:D], b_qkv[D:2 * D]
    in_maps = []
    xTs = [np.ascontiguousarray(x[b].T) for b in range(B)]
    for c in range(N_CORES):
        b, g = divmod(c, GROUPS)
        cs = slice(CS * g, CS * (g + 1))
        in_maps.append({
            "xT": xTs[b],
            "w_qk": np.ascontiguousarray(np.concatenate([Wq[:, cs], Wk[:, cs]], axis=1)),
            "w_v": np.ascontiguousarray(Wv[:, cs]),
            "b_qk": np.ascontiguousarray(
                np.concatenate([bq[cs], bk[cs]]).reshape(2 * CS, 1)),
            "w_out": np.ascontiguousarray(W_out[cs, :]),
        })
    return in_maps


def combine_outputs(results, b_qkv, b_out, W_out):
    b_qkv = np.asarray(b_qkv, dtype=np.float32)
    b_out = np.asarray(b_out, dtype=np.float32)
    W_out = np.asarray(W_out, dtype=np.float32)
    out = np.empty((B, L, D), np.float32)
    for b in range(B):
        acc = results[GROUPS * b]["outT"].astype(np.float32)
        for g in range(1, GROUPS):
            acc = acc + results[GROUPS * b + g]["outT"]
        out[b] = acc.T
    # v-bias folds to a constant row (softmax rows sum to 1); plus b_out.
    bv = b_qkv[2 * D:3 * D]
    out += (bv @ W_out + b_out)[None, None, :]
    return out


def _numpy_reference(x, attention_mask, W_qkv, b_qkv, W_out, b_out):
    x = np.asarray(x, np.float64)
    mask = np.asarray(attention_mask, bool)
    W_qkv = np.asarray(W_qkv, np.float64)
    b_qkv = np.asarray(b_qkv, np.float64)
    W_out = np.asarray(W_out, np.float64)
    b_out = np.asarray(b_out, np.float64)
    Bs, Ls, Ds = x.shape
    qkv = x @ W_qkv + b_qkv
    qkv = qkv.reshape(Bs, Ls, 3, H, HD)
    q = np.transpose(qkv[:, :, 0], (0, 2, 1, 3))
    k = np.transpose(qkv[:, :, 1], (0, 2, 1, 3))
    v = np.transpose(qkv[:, :, 2], (0, 2, 1, 3))
    scores = np.einsum("bhqd,bhkd->bhqk", q, k) / np.sqrt(HD)
    scores = np.where(~mask[:, None, None, :], -np.inf, scores)
    scores = scores - scores.max(axis=-1, keepdims=True)
    attn = np.exp(scores)
    attn = attn / attn.sum(axis=-1, keepdims=True)
    ctx = np.einsum("bhqk,bhkd->bhqd", attn, v)
    ctx = np.transpose(ctx, (0, 2, 1, 3)).reshape(Bs, Ls, Ds)
    return (ctx @ W_out + b_out).astype(np.float32)


def kernel(x, attention_mask, W_qkv, b_qkv, W_out, b_out):
    mask = np.asarray(attention_mask, bool)
    if not mask.all():
        return _numpy_reference(x, attention_mask, W_qkv, b_qkv, W_out, b_out)
    nc = build_nc()
    in_maps = make_in_maps(x, W_qkv, b_qkv, W_out)
    res = run_bass_kernel_spmd(nc, in_maps, list(range(N_CORES)))
    return combine_outputs(res.results, b_qkv, b_out, W_out)


# revision 50
# speedup vs baseline: 1.0445x; 1.0167x over previous
"""Multi-head self-attention (B=2, L=2048, D=1024, H=16) on 8 TRN2 NeuronCores.

Sharding: core c -> (batch b = c//4, head-group g = c%4 of 4 heads).
Each core computes, for its batch element and its 4 heads:
  qkv projection (column-sharded), scores, softmax, attn@V, and the
  row-sharded slice of the output projection (partial sums over D).
Host gathers: sums the 4 partial outputs per batch and transposes.

Device-side design:
  - x is passed pre-transposed (xT [D, L]); q^T and k^T are computed
    directly ([c, L], partition = head channel) so scores^T [k_l, q_l]
    come out of the PE in one pass.
  - ALL matmuls keep K=128: the per-head S^T matmul (d=64) uses a
    zero-padded lhsT (kT2[h]: the head's 64 k-channel rows at their
    natural partition offset, zeros in the other 64 rows) against the
    full 128-partition q^T pair tile.  K<128 matmuls keep the PE's HAM
    clock-gate cold (measured 435 vs 236 ns for N=512) — zero-padding
    keeps the array fully loaded and the clock at 2.4 GHz.
  - exp() without max-subtraction (scores ~N(0,1) after the 1/8 scale,
    folded into the activation scale), over merged [128,1024] tiles to
    amortize ACT instruction overhead.
  - v is augmented with a ones column, so attn@V also yields the
    softmax denominator as row 64 of ctx^T for free.
  - ctx_aug^T [65, q] is PE-transposed (zero-padded to K=128) to
    [q, 65]; normalization is a per-partition reciprocal-scale on DVE.
  - out^T [D, L] = W_out-shard^T @ ctx^T; host transposes + reduces.
  - matmuls run in float32r (full-speed fp32 mode, ~12-bit mantissa);
    every matmul input is produced as f32r (DVE/ACT round on write).
  - q/k biases applied on-device at eviction; the v bias equals adding
    (b_v @ W_out) to the final output (softmax rows sum to 1): host.
"""

import numpy as np
from contextlib import ExitStack

import concourse.bacc as bacc
import concourse.bass as bass
import concourse.tile as tile
from concourse import mybir
from concourse.bass import ts
from concourse.bass_utils import run_bass_kernel_spmd
from concourse.masks import make_identity

# Problem constants (hardcoded per the self-contained-kernel contract).
B, L, D, H, HD = 2, 2048, 1024, 16, 64
N_CORES = 8
GROUPS = 4                  # head-groups per batch element
HPC = H // GROUPS           # heads per core = 4
CS = HPC * HD               # channel shard = 256
P = 128
KT = D // P                 # 8 k-tiles over D
NL = L // 512               # 4 l-chunks of 512
LT = L // P                 # 16 l-tiles of 128
CT_QK = 2 * CS // P         # 4 c-tiles over [q|k] shard (512)

F32 = mybir.dt.float32
F32R = mybir.dt.float32r
USE_F32R = True
MDT = F32R if USE_F32R else F32
Ident = mybir.ActivationFunctionType.Identity
Exp = mybir.ActivationFunctionType.Exp

_NC_CACHE = {}


def _build_body(nc, ctx, tc, xT, w_qk, w_v, b_qk, w_out, outT):
    const = ctx.enter_context(tc.tile_pool(name="const", bufs=1))

    ones_sb = const.tile([P, 1], F32, tag="ones")
    nc.vector.memset(ones_sb[:], 1.0)
    zeros_sb = const.tile([P, 512], F32, tag="zeros")
    nc.vector.memset(zeros_sb[:], 0.0)
    ident = const.tile([P, P], F32, tag="ident")
    make_identity(nc, ident)

    wout_sb = [const.tile([P, D], MDT, tag=f"wout{t}", name=f"wout{t}")
               for t in range(CS // P)]
    bqk_sb = [const.tile([P, 1], F32, tag=f"bqk{m}", name=f"bqk{m}")
              for m in range(CT_QK)]
    # q^T pair tiles: rows 0:64 head 2p, 64:128 head 2p+1
    qT_sb = [const.tile([P, L], MDT, tag=f"qT{p}", name=f"qT{p}") for p in range(2)]
    # zero-padded k^T per head: head rows at natural offset, other half 0
    kT2_sb = [const.tile([P, L], MDT, tag=f"kT2{h}", name=f"kT2{h}") for h in range(HPC)]
    # v_aug per l-tile: per head [v(64) | ones] (65 cols)
    VOFF = [65 * h for h in range(HPC)]
    VTOT = HPC * (HD + 1)
    v_sb = [const.tile([P, VTOT], MDT, tag=f"v{t}", name=f"v{t}") for t in range(LT)]
    ctx_sb = [const.tile([P, CS], F32, tag=f"ctx{t}", name=f"ctx{t}")
              for t in range(LT)]
    cxT_sb = [const.tile([P, L], MDT, tag=f"cxT{t}", name=f"cxT{t}")
              for t in range(CS // P)]

    ptpool = ctx.enter_context(tc.tile_pool(name="pt", bufs=3))

    # stage-1-scoped pools (released after stage 1)
    s1 = ExitStack()
    s1pool = s1.enter_context(tc.tile_pool(name="s1w", bufs=1))
    stage = s1.enter_context(tc.tile_pool(name="stage", bufs=3))
    xpool = s1.enter_context(tc.tile_pool(name="xt", bufs=2))

    pspool = ctx.enter_context(tc.tile_pool(name="ps", bufs=3, space="PSUM"))
    accpool = ctx.enter_context(tc.tile_pool(name="acc", bufs=2, space="PSUM"))

    def load_rounded(dst, src_slice, nm):
        if USE_F32R:
            st = stage.tile(list(dst.shape), F32, tag="st", name=f"st_{nm}")
            nc.sync.dma_start(st[:], src_slice)
            nc.vector.tensor_copy(dst, st[:])
        else:
            nc.sync.dma_start(dst, src_slice)

    wqk_sb = [s1pool.tile([P, 2 * CS], MDT, tag=f"wqk{k}", name=f"wqk{k}")
              for k in range(KT)]
    wv_sb = [s1pool.tile([P, CS], MDT, tag=f"wv{k}", name=f"wv{k}")
             for k in range(KT)]

    # interleave the first l-chunk's x tiles with the qk weights so the
    # first psum accumulation chain can start within a few microseconds
    xts0 = []
    for k in range(KT):
        xt = xpool.tile([P, 512], MDT, tag=f"x{k}", name=f"x{k}_0")
        load_rounded(xt[:], xT[ts(k, P), 0:512], f"x{k}_0")
        xts0.append(xt)
        load_rounded(wqk_sb[k][:], w_qk[ts(k, P), :], f"wqk{k}")
    for m in range(CT_QK):
        nc.sync.dma_start(bqk_sb[m][:], b_qk[ts(m, P), :])
    for k in range(KT):
        load_rounded(wv_sb[k][:], w_v[ts(k, P), :], f"wv{k}")

    # constant fills for kT2 zero halves (gate the first S matmuls)
    for h in range(HPC):
        zr = slice(64, 128) if h % 2 == 0 else slice(0, 64)
        for lc in range(NL):
            nc.vector.tensor_copy(kT2_sb[h][zr, ts(lc, 512)], zeros_sb[0:64, :])

    def make_group(h, qg):
        return {"cps": [accpool.tile([P, 512], F32, tag="acc",
                                     name=f"ctx_ps{h}_{qg}_{i}") for i in range(2)],
                "prev": None, "pt": None}

    def attn_step(g, h, qg, kt):
        sps = pspool.tile([P, 1024], F32, tag="ps", name=f"s_ps{h}_{qg}_{kt}")
        for half in range(2):
            nc.tensor.matmul(sps[:, half * 512:(half + 1) * 512],
                             kT2_sb[h][:, ts(kt, P)],
                             qT_sb[h // 2][:, ts(2 * qg + half, 512)],
                             start=True, stop=True)
        pt = ptpool.tile([P, 1024], MDT, tag="pt", name=f"pt{h}_{qg}_{kt}")
        nc.scalar.activation(pt[:], sps[:], Exp, scale=1.0 / np.sqrt(HD))
        if g["prev"] is not None:
            for half in range(2):
                nc.tensor.matmul(g["cps"][half][0:HD + 1, :],
                                 v_sb[g["prev"]][:, VOFF[h]:VOFF[h] + HD + 1],
                                 g["pt"][:, half * 512:(half + 1) * 512],
                                 start=(g["prev"] == 0), stop=False)
        g["prev"], g["pt"] = kt, pt

    def attn_flush(g, h):
        for half in range(2):
            nc.tensor.matmul(g["cps"][half][0:HD + 1, :],
                             v_sb[g["prev"]][:, VOFF[h]:VOFF[h] + HD + 1],
                             g["pt"][:, half * 512:(half + 1) * 512],
                             start=False, stop=True)

    # ---- Stage 1: qkv projections (all share the x tiles) ----------------
    for lc in range(NL):
        if lc == 0:
            xts = xts0
        else:
            xts = []
            for k in range(KT):
                xt = xpool.tile([P, 512], MDT, tag=f"x{k}", name=f"x{k}_{lc}")
                load_rounded(xt[:], xT[ts(k, P), ts(lc, 512)], f"x{k}_{lc}")
                xts.append(xt)
        for m in range(CT_QK):
            ps = pspool.tile([P, 512], F32, tag="ps", name=f"qk_ps{lc}_{m}")
            for k in range(KT):
                nc.tensor.matmul(ps[:], wqk_sb[k][:, ts(m, P)], xts[k][:],
                                 start=(k == 0), stop=(k == KT - 1))
            if m < 2:
                nc.scalar.activation(qT_sb[m][:, ts(lc, 512)], ps[:], Ident,
                                     bias=bqk_sb[m][:])
            else:
                p = m - 2
                nc.scalar.activation(kT2_sb[2 * p][0:64, ts(lc, 512)], ps[0:64, :],
                                     Ident, bias=bqk_sb[m][0:64, :])
                nc.scalar.activation(kT2_sb[2 * p + 1][64:128, ts(lc, 512)],
                                     ps[64:128, :], Ident, bias=bqk_sb[m][64:128, :])
        for i in range(4):
            t = lc * 4 + i
            vps = pspool.tile([P, CS], F32, tag="ps", name=f"v_ps{t}")
            for k in range(KT):
                nc.tensor.matmul(vps[:], xts[k][:, ts(i, P)], wv_sb[k][:],
                                 start=(k == 0), stop=(k == KT - 1))
            for h in range(HPC):
                # ACT is idle through most of stage 1; keep DVE on the casts
                nc.scalar.copy(v_sb[t][:, VOFF[h]:VOFF[h] + HD],
                               vps[:, ts(h, HD)])
            for h in range(HPC):
                nc.vector.tensor_copy(v_sb[t][:, VOFF[h] + HD:VOFF[h] + HD + 1],
                                      ones_sb[:])
        # interleave head 0 / q-group 0's attention chain into stage 1:
        # its kt range only needs the l-chunks already produced.
        if lc >= 1:
            if lc == 1:
                g0 = make_group(0, 0)
                for kt in range(0, 8):
                    attn_step(g0, 0, 0, kt)
            else:
                for kt in range(4 * lc, 4 * (lc + 1)):
                    attn_step(g0, 0, 0, kt)

    # wout loads (first needed by the deferred out-proj, much later)
    for t in range(CS // P):
        load_rounded(wout_sb[t][:, 0:512], w_out[ts(t, P), 0:512], f"wouta{t}")
        load_rounded(wout_sb[t][:, 512:1024], w_out[ts(t, P), 512:1024], f"woutb{t}")
    s1.close()

    capool = ctx.enter_context(tc.tile_pool(name="ca", bufs=10))
    rpool = ctx.enter_context(tc.tile_pool(name="r", bufs=8))
    opool = ctx.enter_context(tc.tile_pool(name="ot", bufs=4))

    # ---- Stage 2: attention (qg outer); epilogue interleaved --------------
    pending = []   # deferred out-proj emission units, drained 1/kt-iter

    def drain(n):
        for _ in range(min(n, len(pending))):
            pending.pop(0)()

    def outproj_unit(et, lc):
        def emit():
            ops = pspool.tile([P, 512], F32, tag="ps", name=f"o_ps{et}_{lc}")
            for ct in range(CS // P):
                nc.tensor.matmul(ops[:], wout_sb[ct][:, ts(et, P)],
                                 cxT_sb[ct][:, ts(lc, 512)],
                                 start=(ct == 0), stop=(ct == CS // P - 1))
            ot = opool.tile([P, 512], F32, tag="ot", name=f"ot{et}_{lc}")
            nc.vector.tensor_copy(ot[:], ops[:])
            nc.sync.dma_start(outT[ts(et, P), ts(lc, 512)], ot[:])
        return emit

    def tn_unit(h, qc, ca):
        # transpose one 128-block of ctx_aug^T and normalize it into ctx
        def emit():
            tp = pspool.tile([P, 512], F32, tag="ps", name=f"tp{h}_{qc}")
            for i in range(4):
                t = qc * 4 + i
                nc.tensor.transpose(tp[:, ts(i, P)], ca[:, ts(i, P)], ident[:])
            for i in range(4):
                t = qc * 4 + i
                r = rpool.tile([P, 1], F32, tag="r", name=f"r{h}_{qc}_{i}")
                nc.vector.reciprocal(r[:], tp[:, i * P + HD:i * P + HD + 1])
                nc.vector.tensor_scalar_mul(ctx_sb[t][:, ts(h, HD)],
                                            tp[:, i * P:i * P + HD], r[:])
        return emit

    def tp2_unit(qg, ct, j):
        def emit():
            t = 8 * qg + j
            tp2 = pspool.tile([P, 512], F32, tag="ps", name=f"tp2_{ct}_{t}")
            nc.tensor.transpose(tp2[:, 0:P], ctx_sb[t][:, ts(ct, P)], ident[:])
            nc.vector.tensor_copy(cxT_sb[ct][:, ts(t, P)], tp2[:, 0:P])
        return emit

    ca_first = [True] * 10   # zero rows 64:128 on first use of each ca slot
    for qg in range(NL // 2):
        for h in range(HPC):
            if qg == 0 and h == 0:
                g = g0       # computed interleaved with stage 1
            else:
                g = make_group(h, qg)
                for kt in range(LT):
                    attn_step(g, h, qg, kt)
                    drain(1)
            attn_flush(g, h)
            # evict ctx_aug^T to SBUF; defer transpose+normalize
            for half in range(2):
                qc = 2 * qg + half
                ca = capool.tile([P, 512], F32, tag="ca", name=f"ca{h}_{qc}")
                if ca_first:
                    ca_first.pop()
                    nc.vector.tensor_copy(ca[HD:P, :], zeros_sb[HD:P, :])
                nc.vector.tensor_copy(ca[0:HD + 1, :], g["cps"][half][0:HD + 1, :])
                pending.append(tn_unit(h, qc, ca))

        # queue ctx -> ctx^T transposes and out-projection for this q-range
        for ct in range(CS // P):
            for j in range(8):
                pending.append(tp2_unit(qg, ct, j))
        for et in range(D // P):
            for lc in range(2 * qg, 2 * (qg + 1)):
                pending.append(outproj_unit(et, lc))
    drain(len(pending))


def build_nc():
    key = ("v11", USE_F32R)
    if key in _NC_CACHE:
        return _NC_CACHE[key]
    nc = bacc.Bacc("TRN2", target_bir_lowering=False, debug=False)
    xT = nc.dram_tensor("xT", [D, L], F32, kind="ExternalInput").ap()
    w_qk = nc.dram_tensor("w_qk", [D, 2 * CS], F32, kind="ExternalInput").ap()
    w_v = nc.dram_tensor("w_v", [D, CS], F32, kind="ExternalInput").ap()
    b_qk = nc.dram_tensor("b_qk", [2 * CS, 1], F32, kind="ExternalInput").ap()
    w_out = nc.dram_tensor("w_out", [CS, D], F32, kind="ExternalInput").ap()
    outT = nc.dram_tensor("outT", [D, L], F32, kind="ExternalOutput").ap()
    with tile.TileContext(nc) as tc:
        with ExitStack() as ctx:
            _build_body(nc, ctx, tc, xT, w_qk, w_v, b_qk, w_out, outT)
    nc.compile()
    _NC_CACHE[key] = nc
    return nc


def make_in_maps(x, W_qkv, b_qkv, W_out):
    x = np.ascontiguousarray(np.asarray(x, dtype=np.float32))
    W_qkv = np.asarray(W_qkv, dtype=np.float32)
    b_qkv = np.asarray(b_qkv, dtype=np.float32)
    W_out = np.asarray(W_out, dtype=np.float32)
    Wq, Wk, Wv = W_qkv[:, 0:D], W_qkv[:, D:2 * D], W_qkv[:, 2 * D:3 * D]
    bq, bk = b_qkv[0:D], b_qkv[D:2 * D]
    in_maps = []
    xTs = [np.ascontiguousarray(x[b].T) for b in range(B)]
    for c in range(N_CORES):
        b, g = divmod(c, GROUPS)
        cs = slice(CS * g, CS * (g + 1))
        in_maps.append({
            "xT": xTs[b],
            "w_qk": np.ascontiguousarray(np.concatenate([Wq[:, cs], Wk[:, cs]], axis=1)),
            "w_v": np.ascontiguousarray(Wv[:, cs]),
            "b_qk": np.ascontiguousarray(
                np.concatenate([bq[cs], bk[cs]]).reshape(2 * CS, 1)),
            "w_out": np.ascontiguousarray(W_out[cs, :]),
        })
    return in_maps


def combine_outputs(results, b_qkv, b_out, W_out):
    b_qkv = np.asarray(b_qkv, dtype=np.float32)
    b_out = np.asarray(b_out, dtype=np.float32)
    W_out = np.asarray(W_out, dtype=np.float32)
    out = np.empty((B, L, D), np.float32)
    for b in range(B):
        acc = results[GROUPS * b]["outT"].astype(np.float32)
        for g in range(1, GROUPS):
            acc = acc + results[GROUPS * b + g]["outT"]
        out[b] = acc.T
    # v-bias folds to a constant row (softmax rows sum to 1); plus b_out.
    bv = b_qkv[2 * D:3 * D]
    out += (bv @ W_out + b_out)[None, None, :]
    return out


def _numpy_reference(x, attention_mask, W_qkv, b_qkv, W_out, b_out):
    x = np.asarray(x, np.float64)
    mask = np.asarray(attention_mask, bool)
    W_qkv = np.asarray(W_qkv, np.float64)
    b_qkv = np.asarray(b_qkv, np.float64)
    W_out = np.asarray(W_out, np.float64)
    b_out = np.asarray(b_out, np.float64)
    Bs, Ls, Ds = x.shape
    qkv = x @ W_qkv + b_qkv
    qkv = qkv.reshape(Bs, Ls, 3, H, HD)
    q = np.transpose(qkv[:, :, 0], (0, 2, 1, 3))
    k = np.transpose(qkv[:, :, 1], (0, 2, 1, 3))
    v = np.transpose(qkv[:, :, 2], (0, 2, 1, 3))
    scores = np.einsum("bhqd,bhkd->bhqk", q, k) / np.sqrt(HD)
    scores = np.where(~mask[:, None, None, :], -np.inf, scores)
    scores = scores - scores.max(axis=-1, keepdims=True)
    attn = np.exp(scores)
    attn = attn / attn.sum(axis=-1, keepdims=True)
    ctx = np.einsum("bhqk,bhkd->bhqd", attn, v)
    ctx = np.transpose(ctx, (0, 2, 1, 3)).reshape(Bs, Ls, Ds)
    return (ctx @ W_out + b_out).astype(np.float32)


def kernel(x, attention_mask, W_qkv, b_qkv, W_out, b_out):
    mask = np.asarray(attention_mask, bool)
    if not mask.all():
        return _numpy_reference(x, attention_mask, W_qkv, b_qkv, W_out, b_out)
    nc = build_nc()
    in_maps = make_in_maps(x, W_qkv, b_qkv, W_out)
    res = run_bass_kernel_spmd(nc, in_maps, list(range(N_CORES)))
    return combine_outputs(res.results, b_qkv, b_out, W_out)
